# revision 12
# baseline (speedup 1.0000x reference)
"""Feature-pyramid ROIAlign (multi-level crop) on 8 TRN2 NeuronCores — v8.1.

PE band-structure rules honored:
- Slots are split into an UNBANDED region (K>64, PE mode 128x128) and a
  BANDED region (33<=K<=64 after clamping, PE row-tiled mode 64x128).
  Uniform tile_size within each region avoids PE mode-switch drains.
- In banded groups, quads are laid out [b0, b0, b64, b64]: band-0 slots
  share PSUM banks 0-1, band-64 slots banks 2-3 — two row-tiles never
  touch the same PSUM bank (hardware constraint).
- Band-64 slots live at SBUF partitions [64:64+K] (patch + W rows), so
  their matmuls run on PE tile T8 while band-0 runs on T0: LDWEIGHTS of
  one tile overlaps MATMUL of the other.
- One GpSimd indirect DMA gathers a (b0, b64) slot pair (cells at
  partitions [0:K_e] and [64:64+K_o] from one 128-row index column).
- Groups are the pipeline unit: per group — values, W loads (2 for
  banded: rows [0:K_g1] / [64:64+K_g2]), previous group's output store
  (delayed one group so SP never stalls), gathers, matmul quads, PSUM
  drains split DVE/ACT.
"""
import os
import numpy as np
import ml_dtypes

RPN_SCALES = (2.0, 4.0, 8.0, 16.0)
BASE_SIZES = (8.0, 16.0, 32.0, 64.0)
S = 14
S2 = S * S
C = 256
MAP_HW = (256, 128, 64, 32)
ARENA_BASE = (0, 65536, 81920, 86016)  # cell-row base of each level
ARENA_ROWS = 87040
N_CORES = 8
GMAX = 32    # max slots per group (wmat load + output flush unit)
QUAD = 4     # slots per PSUM drain tile (4 banks)
KMIN = 33    # banded slots clamp K to [33, 64] -> uniform 64x128 PE mode

LAST_EXEC_TIME_NS = None
_GRAPH_CACHE = {}


def _route(proposals):
    """Per-proposal level + tight cell rect + bilinear split indices."""
    p = proposals.astype(np.float32)
    x0, y0, x1, y1 = p[:, 1], p[:, 2], p[:, 3], p[:, 4]
    sizes = np.sqrt((x1 - x0) * (y1 - y0))
    base = np.asarray(BASE_SIZES, dtype=np.float32)
    lvl = np.argmin(np.abs(sizes[:, None] - base[None, :]), axis=1).astype(np.int32)

    stride = np.asarray(RPN_SCALES, dtype=np.float32)[lvl]
    M = np.asarray(MAP_HW, dtype=np.int32)[lvl]

    fx0, fy0, fx1, fy1 = (c / stride for c in (x0, y0, x1, y1))
    bw = (fx1 - fx0) / np.float32(S)
    bh = (fy1 - fy0) / np.float32(S)
    grid = np.arange(S, dtype=np.float32) + np.float32(0.5)
    xs = fx0[:, None] + grid[None, :] * bw[:, None] - np.float32(0.5)
    ys = fy0[:, None] + grid[None, :] * bh[:, None] - np.float32(0.5)

    def split(coord, Mv):
        c0 = np.floor(coord)
        frac = coord - c0
        i0 = np.clip(c0.astype(np.int64), 0, Mv - 1).astype(np.int32)
        i1 = np.minimum(i0 + 1, Mv - 1).astype(np.int32)
        return i0, i1, frac.astype(np.float32)

    Mv = M[:, None]
    yi0, yi1, wy = split(ys, Mv)
    xi0, xi1, wx = split(xs, Mv)

    ry0 = yi0.min(axis=1)
    rx0 = xi0.min(axis=1)
    yh = yi1.max(axis=1) - ry0 + 1
    xw = xi1.max(axis=1) - rx0 + 1
    return lvl, M, (yi0, yi1, wy), (xi0, xi1, wx), ry0, rx0, yh, xw


def _shard(lvl, yh, xw):
    """Build the shared slot table.

    Returns slot_gid [8, M] plus per-slot (lvl, yh, xw, band) and the
    group list [(ga, gb, banded), ...]. Banded slots have their shapes
    clamped so K=yh*xw is in [KMIN, 64]."""
    raw = []   # (gid8, lvl, yh, xw)
    for l in range(4):
        ids = np.where(lvl == l)[0]
        if len(ids) == 0:
            continue
        key = yh[ids].astype(np.int64) * 16 + xw[ids]
        ids = ids[np.argsort(-key, kind="stable")]
        pad = (-len(ids)) % N_CORES
        ids = np.concatenate([ids, np.repeat(ids[-1], pad)])
        for j in range(len(ids) // N_CORES):
            g = ids[j * N_CORES:(j + 1) * N_CORES]
            raw.append((g, l, int(yh[g].max()), int(xw[g].max())))

    big = [e for e in raw if e[2] * e[3] > 64]
    mid = [e for e in raw if 32 < e[2] * e[3] <= 64]
    tiny = [e for e in raw if e[2] * e[3] <= 32]
    big.sort(key=lambda e: -(e[2] * e[3]))
    mid.sort(key=lambda e: -(e[2] * e[3]))
    tiny.sort(key=lambda e: -(e[2] * e[3]))
    while len(big) % QUAD:
        big.append(big[-1])
    while len(mid) % (2 * QUAD):
        mid.append(mid[-1])
    while len(tiny) % QUAD:
        tiny.append(tiny[-1])

    slots = []   # (gid8, lvl, yh, xw, band)
    groups = []  # (ga, gb, mode) mode: 0=128x128, 2=64x128, 4=32x128
    for ga in range(0, len(big), GMAX):
        gb = min(ga + GMAX, len(big))
        groups.append((len(slots), len(slots) + gb - ga, 0))
        for e in big[ga:gb]:
            slots.append((*e, 0))
    # 64x128 groups: first half -> band 0, second -> band 64, quads
    # laid out [b0, b0, b64, b64]
    for ga in range(0, len(mid), GMAX):
        gb = min(ga + GMAX, len(mid))
        grp = mid[ga:gb]
        hh = len(grp) // 2
        h1, h2 = grp[:hh], grp[hh:]
        a0 = len(slots)
        for q in range(hh // 2):
            slots.append((*h1[2 * q], 0))
            slots.append((*h1[2 * q + 1], 0))
            slots.append((*h2[2 * q], 64))
            slots.append((*h2[2 * q + 1], 64))
        groups.append((a0, len(slots), 2))
    # 32x128 groups: quad = one slot per 32-row band (0/32/64/96);
    # band u across the group = sorted slice [u*nq : (u+1)*nq]
    for ga in range(0, len(tiny), GMAX):
        gb = min(ga + GMAX, len(tiny))
        grp = tiny[ga:gb]
        nqg = len(grp) // QUAD
        a0 = len(slots)
        for q in range(nqg):
            for u in range(QUAD):
                slots.append((*grp[u * nqg + q], 32 * u))
        groups.append((a0, len(slots), 4))

    slot_gid = np.asarray([e[0] for e in slots]).T
    slot_lvl = tuple(e[1] for e in slots)
    slot_yh = tuple(e[2] for e in slots)
    slot_xw = tuple(e[3] for e in slots)
    band = tuple(e[4] for e in slots)
    return slot_gid, slot_lvl, slot_yh, slot_xw, band, tuple(groups)


def _wcol(p, mode, gs):
    """wmat column (within group of size gs) for slot position p.
    mode 2 stores b0-half columns first; mode 4 stores columns grouped
    by band (band u columns = [u*nq .. u*nq+nq))."""
    if mode == 0:
        return p
    if mode == 2:
        u, w = p // 4, p % 4
        if w < 2:
            return u * 2 + w
        return gs // 2 + u * 2 + (w - 2)
    nqg = gs // QUAD
    return (p % 4) * nqg + p // 4


def _assign_engines(sig):
    """Per-slot gather engine: 0=SP dyn, 1=ACT dyn, 2=Q7 paired
    indirect (pairs are (q0+0, q0+2) and (q0+1, q0+3) of banded
    quads). Per-quad drain engine: 0=DVE, 1=ACT."""
    slot_lvl, slot_yh, slot_xw, band, groups = sig
    M = len(slot_lvl)
    K_of = [slot_yh[j] * slot_xw[j] for j in range(M)]
    nq = M // QUAD
    dve_l, act_dr = 0.0, 22.0
    drain = []
    for q in range(nq):
        if act_dr + 1.57 <= dve_l + 1.76:
            drain.append(1)
            act_dr += 1.57
        else:
            drain.append(0)
            dve_l += 1.76
    act_drain_us = sum(1.57 for d in drain if d)
    load = {0: 9.0, 1: 9.0 + act_drain_us, 2: 2.0}
    cost = {0: 0.88, 1: 0.88}

    def dyn(j):
        e = 0 if load[0] + cost[0] <= load[1] + cost[1] else 1
        eng[j] = e
        load[e] += cost[e]

    eng = [None] * M
    for ga, gb, mode in groups:
        if mode == 4:
            for q0 in range(ga, gb, QUAD):
                # whole quad in one Q7 DMA vs four dyn DMAs
                if load[2] + 1.28 <= max(load[0], load[1]) + 2.6:
                    for d in range(QUAD):
                        eng[q0 + d] = 2
                    load[2] += 1.28
                else:
                    for d in range(QUAD):
                        dyn(q0 + d)
        elif mode == 2:
            for q0 in range(ga, gb, QUAD):
                for d in (0, 1):
                    # pair (q0+d, q0+d+2): one Q7 DMA vs two dyn DMAs
                    if load[2] + 1.28 <= max(load[0], load[1]) + 1.4:
                        eng[q0 + d] = eng[q0 + d + 2] = 2
                        load[2] += 1.28
                    else:
                        dyn(q0 + d)
                        dyn(q0 + d + 2)
        else:
            for j in range(ga, gb):
                dyn(j)
    return tuple(eng), tuple(drain)


def _build_graph(sig):
    import concourse.bass as bass
    import concourse.bacc as bacc
    import concourse.mybir as mybir
    import concourse.tile as tile

    slot_lvl, slot_yh, slot_xw, band, groups = sig
    M = len(slot_lvl)
    eng, drain = _assign_engines(sig)
    K_of = [slot_yh[j] * slot_xw[j] for j in range(M)]
    KMAX = max(K_of)
    sp_slots = [j for j in range(M) if eng[j] == 0]
    act_slots = [j for j in range(M) if eng[j] == 1]
    q7_firsts = [j for j in range(M) if eng[j] == 2 and band[j] == 0]
    n_sp, n_act = len(sp_slots), len(act_slots)
    sp_pos = {j: i for i, j in enumerate(sp_slots)}
    act_pos = {j: i for i, j in enumerate(act_slots)}
    pair_col = {j: i for i, j in enumerate(q7_firsts)}

    SP = (mybir.EngineType.SP,)
    ACT = (mybir.EngineType.Activation,)
    nc = bacc.Bacc()
    arena = nc.declare_dram_parameter("arena", [ARENA_ROWS, C],
                                      mybir.dt.bfloat16, isOutput=False)
    lvl_view = [arena[ARENA_BASE[l]:ARENA_BASE[l] + MAP_HW[l] * MAP_HW[l], :]
                for l in range(4)]
    wmat = nc.declare_dram_parameter("wmat", [KMAX, M, S2], mybir.dt.bfloat16,
                                     isOutput=False)
    orig = nc.declare_dram_parameter("orig", [1, max(n_sp + n_act, 1)],
                                     mybir.dt.int32, isOutput=False)
    idxg = nc.declare_dram_parameter("idxg", [128, max(len(q7_firsts), 1)],
                                     mybir.dt.int32, isOutput=False)
    out = nc.declare_dram_parameter("out", [C, M, S2], mybir.dt.bfloat16,
                                    isOutput=True)

    with tile.TileContext(nc) as tc:
        with (
            tc.tile_pool(name="small", bufs=1) as psmall,
            tc.tile_pool(name="wpool", bufs=4) as pwp,
            tc.tile_pool(name="patch", bufs=44) as pp,
            tc.tile_pool(name="outp", bufs=4) as po,
            tc.tile_pool(name="ps", bufs=2, space="PSUM") as ppsum,
        ):
            orig_t = psmall.tile([1, max(n_sp + n_act, 1)], mybir.dt.int32)
            nc.sync.dma_start(orig_t[:], orig[:])
            idxg_t = psmall.tile([128, max(len(q7_firsts), 1)],
                                 mybir.dt.int32)
            nc.scalar.dma_start(idxg_t[:], idxg[:])

            # all dyn offsets loaded up front, <=20 registers per call
            def load_vals(base, n, engines):
                vals = []
                for o in range(0, n, 20):
                    _, v = nc.values_load_multi_w_load_instructions(
                        orig_t[0:1, base + o:base + min(o + 20, n)],
                        engines=engines, skip_runtime_bounds_check=True)
                    vals.extend(v)
                return tuple(vals)

            vals_sp = load_vals(0, n_sp, SP) if sp_slots else ()
            vals_act = load_vals(n_sp, n_act, ACT) if act_slots else ()

            pend_outs = []
            for gi, (a, b, mode) in enumerate(groups):
                gs = b - a
                wt = pwp.tile([128, GMAX * S2], mybir.dt.bfloat16, tag="wt")
                if mode == 0:
                    nbands, rows = 1, 128
                else:
                    nbands, rows = mode // 2 * 2, 128 // (mode // 2 * 2)
                ncols = gs // nbands
                for u in range(nbands):
                    K_gu = max(K_of[j] for j in range(a, b)
                               if band[j] == u * rows)
                    nc.scalar.dma_start(
                        wt[u * rows:u * rows + K_gu,
                           u * ncols * S2:(u + 1) * ncols * S2].rearrange(
                            "k (p n) -> k p n", p=ncols),
                        wmat[0:K_gu, a + u * ncols:a + (u + 1) * ncols, :])

                if len(pend_outs) >= 2:
                    pa, pb, pgs, poutAB = pend_outs.pop(0)
                    nc.sync.dma_start(out[0:128, pa:pb, :],
                                      poutAB[:, 0:pgs * S2])
                    nc.scalar.dma_start(
                        out[128:256, pa:pb, :],
                        poutAB[:, GMAX * S2:GMAX * S2 + pgs * S2])

                # patch gathers
                pts = {}
                for j in range(a, b):
                    if j in pts:
                        continue
                    l = slot_lvl[j]
                    Wl = MAP_HW[l]
                    K = K_of[j]
                    bd = band[j]
                    if eng[j] == 2:
                        if bd != 0:
                            continue  # handled by its group key slot
                        if mode == 4:
                            mates = [j, j + 1, j + 2, j + 3]
                            span = 96 + K_of[j + 3]
                        else:
                            mates = [j, j + 2]
                            span = 64 + K_of[j + 2]
                        pt = pp.tile([128, C], mybir.dt.bfloat16, tag="pt")
                        c0 = pair_col[j]
                        nc.gpsimd.indirect_dma_start(
                            out=pt[0:span, :],
                            out_offset=None,
                            in_=arena[:],
                            in_offset=bass.IndirectOffsetOnAxis(
                                ap=idxg_t[0:span, c0:c0 + 1], axis=0),
                        )
                        for jm in mates:
                            pts[jm] = pt
                        continue
                    pt = pp.tile([128, C], mybir.dt.bfloat16, tag="pt")
                    if eng[j] == 0:
                        e = nc.sync
                        comb = vals_sp[sp_pos[j]]
                    else:
                        e = nc.scalar
                        comb = vals_act[act_pos[j]]
                    src = (lvl_view[l][bass.ds(comb, slot_yh[j] * Wl), :]
                           .rearrange("(y w) c -> y w c", w=Wl)
                           [:, 0:slot_xw[j], :])
                    e.dma_start(pt[bd:bd + K, :], src)
                    pts[j] = pt

                outAB = po.tile([128, 2 * GMAX * S2], mybir.dt.bfloat16,
                                tag="outAB")
                for q0 in range(a, b, QUAD):
                    # one PSUM bank per slot: [A(s) B(s) gap] in bank s
                    ps = ppsum.tile([128, 2048], mybir.dt.float32, tag="ps")
                    for dq in range(QUAD):
                        jj = q0 + dq
                        K = K_of[jj]
                        bd = band[jj]
                        pt = pts[jj]
                        wc = _wcol(jj - a, mode, gs)
                        sl_w = slice(wc * S2, (wc + 1) * S2)
                        colA = dq * 512
                        colB = colA + S2
                        tp = None if mode == 0 else (bd, 0)
                        nc.tensor.matmul(ps[:, colA:colA + S2],
                                         pt[bd:bd + K, 0:128],
                                         wt[bd:bd + K, sl_w],
                                         start=True, stop=True,
                                         tile_position=tp)
                        nc.tensor.matmul(ps[:, colB:colB + S2],
                                         pt[bd:bd + K, 128:256],
                                         wt[bd:bd + K, sl_w],
                                         start=True, stop=True,
                                         tile_position=tp)
                    src = ps[:].rearrange("p (s n) -> p s n", s=4)[
                        :, :, 0:2 * S2].rearrange("p s (h n) -> p s h n", h=2)
                    dst = outAB[:].rearrange("p (h s n) -> p h s n", h=2,
                                             s=GMAX)[
                        :, :, q0 - a:q0 - a + QUAD, :].rearrange(
                        "p h s n -> p s h n")
                    if drain[q0 // QUAD]:
                        nc.scalar.copy(dst, src)
                    else:
                        nc.vector.tensor_copy(dst, src)
                pend_outs.append((a, b, gs, outAB))

            for pa, pb, pgs, poutAB in pend_outs:
                nc.sync.dma_start(out[0:128, pa:pb, :],
                                  poutAB[:, 0:pgs * S2])
                nc.scalar.dma_start(out[128:256, pa:pb, :],
                                  poutAB[:, GMAX * S2:GMAX * S2 + pgs * S2])
    nc.finalize()
    return nc


def _prep_core_inputs(k, slot_gid, sig, lvl, splits_y, splits_x, ry0, rx0):
    """Per-core tables. wmat columns within banded groups are permuted
    b0-half-first (see _wcol)."""
    slot_lvl, slot_yh, slot_xw, band, groups = sig
    M = len(slot_lvl)
    eng, _ = _assign_engines(sig)
    K_of = [slot_yh[j] * slot_xw[j] for j in range(M)]
    KMAX = max(K_of)
    yi0, yi1, wy = splits_y
    xi0, xi1, wx = splits_x
    gids = slot_gid[k]
    grp_of = {}
    for gi, (a, b, mode) in enumerate(groups):
        for j in range(a, b):
            grp_of[j] = (a, b, mode)

    wm = np.zeros((KMAX, M, S2), dtype=np.float32)
    comb = np.zeros(M, dtype=np.int64)
    ii = np.arange(S)
    for j in range(M):
        g = gids[j]
        Wl = MAP_HW[slot_lvl[j]]
        syh, sxw = slot_yh[j], slot_xw[j]
        oy = min(int(ry0[g]), Wl - syh)
        ox = min(int(rx0[g]), Wl - sxw)
        comb[j] = oy * Wl + ox
        ly0, ly1 = yi0[g] - oy, yi1[g] - oy
        lx0, lx1 = xi0[g] - ox, xi1[g] - ox
        assert ly0.min() >= 0 and ly1.max() < syh, (j, g, syh, ly0, ly1)
        assert lx0.min() >= 0 and lx1.max() < sxw, (j, g, sxw, lx0, lx1)
        Wy = np.zeros((S, syh), dtype=np.float32)
        Wx = np.zeros((S, sxw), dtype=np.float32)
        np.add.at(Wy, (ii, ly0), 1.0 - wy[g])
        np.add.at(Wy, (ii, ly1), wy[g])
        np.add.at(Wx, (ii, lx0), 1.0 - wx[g])
        np.add.at(Wx, (ii, lx1), wx[g])
        a, b, mode = grp_of[j]
        wc = a + _wcol(j - a, mode, b - a)
        wm[0:K_of[j], wc, :] = np.einsum(
            "iy,jx->yxij", Wy, Wx).reshape(K_of[j], S2)

    sp_slots = [j for j in range(M) if eng[j] == 0]
    act_slots = [j for j in range(M) if eng[j] == 1]
    q7_firsts = [j for j in range(M) if eng[j] == 2 and band[j] == 0]
    og = np.concatenate([comb[sp_slots], comb[act_slots]]) if (
        sp_slots or act_slots) else np.zeros(1)
    og = np.ascontiguousarray(og.reshape(1, -1).astype(np.int32))

    def cell_rows(j):
        Wl = MAP_HW[slot_lvl[j]]
        syh, sxw = slot_yh[j], slot_xw[j]
        dy = np.repeat(np.arange(syh), sxw)
        dx = np.tile(np.arange(sxw), syh)
        return (ARENA_BASE[slot_lvl[j]] + (comb[j] // Wl + dy) * Wl
                + (comb[j] % Wl + dx))

    ig = np.zeros((128, max(len(q7_firsts), 1)), dtype=np.int64)
    for i, j in enumerate(q7_firsts):
        mode = grp_of[j][2]
        if mode == 4:
            for u in range(QUAD):
                ig[32 * u:32 * u + K_of[j + u], i] = cell_rows(j + u)
        else:
            ig[0:K_of[j], i] = cell_rows(j)
            ig[64:64 + K_of[j + 2], i] = cell_rows(j + 2)
    idxg = np.ascontiguousarray(ig.astype(np.int32))
    wm_bf = wm.astype(ml_dtypes.bfloat16)
    return wm_bf, og, idxg


def _install_profile_hook():
    """Register the NTFF profile hook (ctypes into libaxon_pjrt.so) so
    run_bass_kernel_spmd(trace=True) can report exec_time_ns under axon."""
    import contextlib
    import ctypes
    import sys
    import types
    if "antenv.axon_hooks" in sys.modules:
        return
    so_path = "/opt/axon/libaxon_pjrt.so"
    try:
        lib = ctypes.CDLL(so_path)
        lib.axon_start_nrt_profile.argtypes = [
            ctypes.POINTER(ctypes.c_int64), ctypes.c_size_t]
        lib.axon_start_nrt_profile.restype = ctypes.c_int64
        lib.axon_stop_nrt_profile.argtypes = [ctypes.c_char_p]
        lib.axon_stop_nrt_profile.restype = ctypes.c_int64
    except (OSError, AttributeError):
        return

    @contextlib.contextmanager
    def _hook(output_dir, device_ids):
        import jax
        jax.devices()
        if device_ids:
            ids = (ctypes.c_int64 * len(device_ids))(*device_ids)
            rc = lib.axon_start_nrt_profile(ids, len(device_ids))
        else:
            rc = lib.axon_start_nrt_profile(None, 0)
        if rc != 0:
            raise RuntimeError(f"axon_start_nrt_profile rc={rc}")
        try:
            yield
        finally:
            n = lib.axon_stop_nrt_profile(str(output_dir).encode())
            if n < 0:
                raise RuntimeError(f"axon_stop_nrt_profile rc={n}")

    mod = types.ModuleType("antenv.axon_hooks")
    mod.get_axon_ntff_profile_hook = lambda: _hook
    mod.set_axon_ntff_profile_hook = lambda h: None
    sys.modules["antenv.axon_hooks"] = mod
    try:
        import antenv
        antenv.axon_hooks = mod
    except ImportError:
        pass


def kernel(f0, f1, f2, f3, proposals):
    global LAST_EXEC_TIME_NS
    try:
        _install_profile_hook()
    except Exception:
        pass
    from concourse.bass_utils import run_bass_kernel_spmd

    feats = (f0, f1, f2, f3)
    N = proposals.shape[0]
    lvl, _, splits_y, splits_x, ry0, rx0, yh, xw = _route(np.asarray(proposals))
    slot_gid, slot_lvl, slot_yh, slot_xw, band, groups = _shard(lvl, yh, xw)
    sig = (slot_lvl, slot_yh, slot_xw, band, groups)
    M = len(slot_lvl)

    if sig not in _GRAPH_CACHE:
        _GRAPH_CACHE[sig] = _build_graph(sig)
    nc = _GRAPH_CACHE[sig]

    arena_np = np.concatenate([
        np.ascontiguousarray(np.asarray(f)[0].transpose(1, 2, 0)).astype(
            ml_dtypes.bfloat16).reshape(-1, C)
        for f in feats
    ], axis=0)
    assert arena_np.shape[0] == ARENA_ROWS

    in_maps = []
    for k in range(N_CORES):
        wm, og, idxg = _prep_core_inputs(
            k, slot_gid, sig, lvl, splits_y, splits_x, ry0, rx0)
        in_maps.append({"arena": arena_np, "wmat": wm, "orig": og,
                        "idxg": idxg})

    trace = os.environ.get("KERNEL_TRACE", "0") == "1"
    res = run_bass_kernel_spmd(nc, in_maps, list(range(N_CORES)), trace=trace)
    LAST_EXEC_TIME_NS = res.exec_time_ns

    out_full = np.zeros((N, C, S2), dtype=np.float32)
    for k in range(N_CORES):
        out_full[slot_gid[k]] = res.results[k]["out"].astype(
            np.float32).transpose(1, 0, 2)
    return out_full.reshape(N, C, S, S)
